# revision 1
# baseline (speedup 1.0000x reference)
"""MultiHeadCrossSimilarity Trainium2 kernel (8 NeuronCores, SPMD).

Math (reference): x [1, N=8192, E=1024] -> q = x@Wq.T+bq, k = x@Wk.T+bk,
reshaped to [G=16, B=512, h=2, d=512]. For every ordered group pair (g, j)
and head h: sim block S = q_g @ k_j.T [B, B]; inner stats over partner
boxes: mean/max/min -> [G, G, 3, B, h]; outer stats over partner groups
j != g: avg/max/min -> [G, 3, 3, B, h] -> output [G*B, 18].

Sharding (8 cores = 2 heads x 4 j-blocks): core c handles head h = c//4
and partner groups Jset = [4*jb, 4*jb+4), jb = c%4. Each core computes
qT for ALL 16 query groups (its head) and kT for its 4 partner groups,
then emits RAW inner stats [8192, 4, 3] (mean/max/min over the 512 boxes
of each of its 4 partner groups, for every query box). The host gathers
32 such partials and performs the outer (over-j, diag-excluded) combine
plus output interleave — cheap [8192, 16, 3]-sized numpy work.

All matmuls run in float32r (full-rate reduced-precision fp32, ~1e-4
rel err — 16x better than bf16 at the same speed). The inner mean is
computed exactly via linearity: mean_k(q . k) = q . ksum_j / B, with a
tiny [128 x 4] matmul against per-group k sums.

The per-instruction sync-wait splitter (wait_split) works around this
container's walrus build accepting at most one embedded sync wait.
"""

import numpy as np

import concourse.bass as bass
import concourse.mybir as mybir
import concourse.tile as tile
from concourse.bass_utils import run_bass_kernel_spmd
from concourse.masks import make_identity

G = 16          # groups (images)
B = 512         # boxes per group
N = G * B       # 8192
E = 1024        # embedding
D = 512         # head dim (h = 2 heads, E = 2*D)
NCORES = 8
JB = 4          # partner groups per core
AF = mybir.ActivationFunctionType
X_AXIS = mybir.AxisListType.X


def _split_multi_waits(nc):
    """walrus here accepts <=1 embedded sync wait per instruction; move
    extras onto same-engine NoOps placed immediately before (engine
    program order makes this equivalent)."""
    ctr = 0
    for f in nc.m.functions:
        for b in f.blocks:
            new_insts = []
            for inst in b.instructions:
                si = getattr(inst, "sync_info", None)
                ow = list(si.on_wait) if (si is not None and si.on_wait) else []
                if len(ow) > 1:
                    for w in ow[:-1]:
                        ctr += 1
                        nop = mybir.InstNoOp(name=f"wsplit-{ctr}", ins=[], outs=[])
                        nop.engine = inst.engine
                        nop.sync_info = mybir.SyncInfo(on_wait=[w], on_update=[])
                        new_insts.append(nop)
                    inst.sync_info = mybir.SyncInfo(
                        on_wait=[ow[-1]], on_update=list(si.on_update or [])
                    )
                new_insts.append(inst)
            b.instructions[:] = new_insts


def _build():
    f32 = mybir.dt.float32
    f32r = mybir.dt.float32r

    nc = bass.Bass("TRN2", target_bir_lowering=False, debug=False,
                   num_devices=NCORES)
    x = nc.dram_tensor("x", [N, E], f32r, kind="ExternalInput").ap()
    xk = nc.dram_tensor("xk", [JB * B, E], f32r, kind="ExternalInput").ap()
    wqt = nc.dram_tensor("wqt", [E, D], f32r, kind="ExternalInput").ap()
    wkt = nc.dram_tensor("wkt", [E, D], f32r, kind="ExternalInput").ap()
    bq = nc.dram_tensor("bq", [D], f32, kind="ExternalInput").ap()
    bk = nc.dram_tensor("bk", [D], f32, kind="ExternalInput").ap()
    out = nc.dram_tensor("out", [N, JB, 3], f32, kind="ExternalOutput").ap()

    with tile.TileContext(nc) as tc:
        with (
            tc.tile_pool(name="wpool", bufs=1) as wpool,
            tc.tile_pool(name="kpool", bufs=1) as kpool,
            tc.tile_pool(name="xin", bufs=2) as xin,
            tc.tile_pool(name="xtst", bufs=2) as xtst,
            tc.tile_pool(name="qpool", bufs=2) as qpool,
            tc.tile_pool(name="spool", bufs=2) as spool,
            tc.tile_pool(name="misc", bufs=1) as misc,
            tc.tile_pool(name="ps_t", bufs=1, space="PSUM") as ps_t,
            tc.tile_pool(name="ps_p", bufs=1, space="PSUM") as ps_p,
            tc.tile_pool(name="ps_s", bufs=2, space="PSUM") as ps_s,
            tc.tile_pool(name="ps_m", bufs=1, space="PSUM") as ps_m,
        ):
            # ---- weights / constants -------------------------------------
            wq_sb = wpool.tile([128, 8, D], f32r, tag="wq")
            wk_sb = wpool.tile([128, 8, D], f32r, tag="wk")
            nc.sync.dma_start(out=wq_sb, in_=wqt.rearrange("(c p) d -> p c d", p=128))
            nc.sync.dma_start(out=wk_sb, in_=wkt.rearrange("(c p) d -> p c d", p=128))
            bq_sb = wpool.tile([128, 4], f32, tag="bq")
            bk_sb = wpool.tile([128, 4], f32, tag="bk")
            nc.sync.dma_start(out=bq_sb, in_=bq.rearrange("(c p) -> p c", p=128))
            nc.sync.dma_start(out=bk_sb, in_=bk.rearrange("(c p) -> p c", p=128))
            ident_f = misc.tile([128, 128], f32, tag="ident_f")
            make_identity(nc, ident_f)
            ident = misc.tile([128, 128], f32r, tag="ident")
            nc.vector.tensor_copy(ident, ident_f)

            kT = kpool.tile([128, 4, JB * B], f32r, tag="kT")   # [dpart, dchunk, n]
            ksum_f = misc.tile([128, 4, JB], f32, tag="ksum_f")
            ksum = misc.tile([128, 4, JB], f32r, tag="ksum")
            scratch = misc.tile([128, B], f32, tag="scratch")

            def load_span(src_rows):
                """DMA 512 input rows -> [128, 4(tt), 1024(e)] f32r tile."""
                t = xin.tile([128, 4, E], f32r, tag="xspan")
                nc.sync.dma_start(
                    out=t, in_=src_rows.rearrange("(tt p) e -> p tt e", p=128)
                )
                return t

            def transpose_span(xt):
                """[128, 4, 1024] span -> xT staging [128, 8(echunk), 512(n)]."""
                st = xtst.tile([128, 8, B], f32r, tag="xT")
                for half in range(4):
                    tp = ps_t.tile([128, 2, B], f32r, tag="tp")
                    for tt in range(4):
                        for mi in range(2):
                            m = half * 2 + mi
                            nc.tensor.transpose(
                                tp[:, mi, tt * 128:(tt + 1) * 128],
                                xt[:, tt, m * 128:(m + 1) * 128],
                                ident,
                            )
                    nc.scalar.activation(
                        st[:, half * 2:(half + 1) * 2, :], tp, AF.Identity
                    )
                return st

            def project(st, w_sb, b_sb, dst3, n0):
                """xT staging + weights -> dst3[:, dm, n0:n0+512] (+bias)."""
                for dm in range(4):
                    pp = ps_p.tile([128, B], f32, tag="pp")
                    for e in range(8):
                        nc.tensor.matmul(
                            pp,
                            w_sb[:, e, dm * 128:(dm + 1) * 128],
                            st[:, e, :],
                            start=(e == 0),
                            stop=(e == 7),
                        )
                    nc.scalar.activation(
                        dst3[:, dm, n0:n0 + B], pp, AF.Identity,
                        bias=b_sb[:, dm:dm + 1],
                    )

            # ---- phase K: kT for the core's 4 partner groups -------------
            for js in range(JB):
                xt = load_span(xk[js * B:(js + 1) * B, :])
                st = transpose_span(xt)
                project(st, wk_sb, bk_sb, kT, js * B)

            # per-group k sums (for the exact inner mean via linearity)
            for dm in range(4):
                for j in range(JB):
                    nc.scalar.activation(
                        scratch, kT[:, dm, j * B:(j + 1) * B], AF.Identity,
                        accum_out=ksum_f[:, dm, j:j + 1],
                    )
            nc.vector.tensor_copy(ksum, ksum_f)

            # ---- phase G: per query group, project q and emit stats ------
            for g in range(G):
                xt = load_span(x[g * B:(g + 1) * B, :])
                st = transpose_span(xt)
                qg = qpool.tile([128, 4, B], f32r, tag="qg")
                project(st, wq_sb, bq_sb, qg, 0)

                stats = spool.tile([128, 4, JB, 3], f32, tag="stats")
                for qt in range(4):
                    lhs = [qg[:, dc, qt * 128:(qt + 1) * 128] for dc in range(4)]
                    # inner mean: (q . ksum_j) / B
                    mp = ps_m.tile([128, JB], f32, tag="mp")
                    for dc in range(4):
                        nc.tensor.matmul(mp, lhs[dc], ksum[:, dc, :],
                                         start=(dc == 0), stop=(dc == 3))
                    nc.scalar.activation(
                        stats[:, qt, :, 0], mp, AF.Identity, scale=1.0 / B
                    )
                    # inner max / min over each partner group's 512 boxes
                    for jc in range(2):
                        sp = ps_s.tile([128, 2, B], f32, tag="sp")
                        for jj in range(2):
                            j = jc * 2 + jj
                            for dc in range(4):
                                nc.tensor.matmul(
                                    sp[:, jj, :], lhs[dc],
                                    kT[:, dc, j * B:(j + 1) * B],
                                    start=(dc == 0), stop=(dc == 3),
                                )
                        nc.vector.tensor_reduce(
                            stats[:, qt, jc * 2:(jc + 1) * 2, 1], sp,
                            axis=X_AXIS, op=mybir.AluOpType.max,
                        )
                        nc.vector.tensor_reduce(
                            stats[:, qt, jc * 2:(jc + 1) * 2, 2], sp,
                            axis=X_AXIS, op=mybir.AluOpType.min,
                        )
                nc.sync.dma_start(
                    out=out[g * B:(g + 1) * B]
                    .rearrange("(qt p) j m -> p qt j m", p=128),
                    in_=stats,
                )

    _split_multi_waits(nc)
    return nc


_NC_CACHE = None


def _get_nc():
    global _NC_CACHE
    if _NC_CACHE is None:
        _NC_CACHE = _build()
    return _NC_CACHE


def kernel(x, Wq, bq, Wk, bk, eachimg_selected_box_nums):
    x = np.ascontiguousarray(np.asarray(x, dtype=np.float32).reshape(N, E))
    Wq = np.asarray(Wq, dtype=np.float32)
    Wk = np.asarray(Wk, dtype=np.float32)
    bq = np.asarray(bq, dtype=np.float32)
    bk = np.asarray(bk, dtype=np.float32)

    nc = _get_nc()
    in_maps = []
    for c in range(NCORES):
        h, jb = divmod(c, JB)
        sl = slice(h * D, (h + 1) * D)
        in_maps.append({
            "x": x,
            "xk": np.ascontiguousarray(x[jb * JB * B:(jb + 1) * JB * B, :]),
            "wqt": np.ascontiguousarray(Wq[sl, :].T),
            "wkt": np.ascontiguousarray(Wk[sl, :].T),
            "bq": np.ascontiguousarray(bq[sl]),
            "bk": np.ascontiguousarray(bk[sl]),
        })
    res = run_bass_kernel_spmd(nc, in_maps, core_ids=list(range(NCORES)))

    # host combine: outer reductions over partner groups j != g
    final = np.empty((N, 18), dtype=np.float32)
    for h in range(2):
        stats = np.concatenate(
            [res.results[h * JB + jb]["out"] for jb in range(JB)], axis=1
        )  # [N, 16, 3] (j, m=mean/max/min)
        for g in range(G):
            rows = slice(g * B, (g + 1) * B)
            sub = np.delete(stats[rows], g, axis=1)  # [B, 15, 3]
            outer = np.stack(
                [sub.mean(1), sub.max(1), sub.min(1)], axis=1
            )  # [B, 3(n), 3(m)]
            final[rows, h::2] = outer.reshape(B, 9)
    return final


# revision 7
# speedup vs baseline: 148.5100x; 148.5100x over previous
"""MultiHeadCrossSimilarity Trainium2 kernel (8 NeuronCores, SPMD).

Math (reference): x [1, N=8192, E=1024] -> q = x@Wq.T+bq, k = x@Wk.T+bk,
reshaped to [G=16, B=512, h=2, d=512]. For every ordered group pair (g, j)
and head h: sim block S = q_g @ k_j.T [B, B]; inner stats over partner
boxes: mean/max/min -> [G, G, 3, B, h]; outer stats over partner groups
j != g: avg/max/min -> [G, 3, 3, B, h] -> output [G*B, 18].

Sharding (8 cores = 2 heads x 4 partner-group blocks): core c handles
head h = c//4 and partner groups Jset = [4*jb, 4*jb+4), jb = c%4. Each
core projects kT for its 4 partner groups (from its x row-slice input
"xk") and, streaming over all 16 query groups g, projects qT_g from the
full x and computes raw inner stats: mean/max/min over the 512 boxes of
each of its 4 partner groups, for every query box -> output [8192, 4, 3]
per core. The host gathers the 8 partials and performs the outer
(over-j, diagonal-excluded) combine plus the head interleave — cheap
[8192, 16, 3]-sized numpy work. This keeps the SPMD instruction stream
identical across cores (per-core behavior differs only through input
data) and needs no cross-core communication.

All matmuls run in float32r (full-rate reduced-precision fp32 at
1 cyc/row for N=512; measured ~1.4e-4 rel-to-absmax error on K=512 dots
— 16x better than bf16 at the same speed). x is transposed on the
tensor engine (f32r transpose, 1.5 cyc/row) because the matmul
contraction dim must live on partitions. The inner mean is computed
exactly via linearity: mean_k(q . k) = q . ksum_j / B, a tiny [128 x 4]
matmul against per-group k sums (ACT accumulates ksum from kT for free
during a copy pass).

Engine budget per core (cost-model validated): PE ~408 us (960 sim
matmuls + projections + transposes), DVE ~305 us (the hard floor: one
max pass + one min pass over every similarity element, tensor_reduce is
1x-mode only), ACT ~157 us (PSUM evictions with fused bias/scale), DMA
~132 us — total ~476 us, PE-bound with DVE close behind.

`_split_multi_waits` works around this container's walrus build
accepting at most ONE embedded sync wait per instruction: extra waits
move onto same-engine NoOps placed immediately before the instruction
(engine program order makes this equivalent).
"""

import numpy as np

import concourse.bass as bass
import concourse.mybir as mybir
import concourse.tile as tile
from concourse.bass_utils import run_bass_kernel_spmd
from concourse.masks import make_identity

G = 16          # groups (images)
B = 512         # boxes per group
N = G * B       # 8192
E = 1024        # embedding
D = 512         # head dim (h = 2 heads, E = 2*D)
NCORES = 8
JB = 4          # partner groups per core
AF = mybir.ActivationFunctionType
X_AXIS = mybir.AxisListType.X


def _split_multi_waits(nc):
    ctr = 0
    for f in nc.m.functions:
        for b in f.blocks:
            new_insts = []
            for inst in b.instructions:
                si = getattr(inst, "sync_info", None)
                ow = list(si.on_wait) if (si is not None and si.on_wait) else []
                if len(ow) > 1:
                    for w in ow[:-1]:
                        ctr += 1
                        nop = mybir.InstNoOp(name=f"wsplit-{ctr}", ins=[], outs=[])
                        nop.engine = inst.engine
                        nop.sync_info = mybir.SyncInfo(on_wait=[w], on_update=[])
                        new_insts.append(nop)
                    inst.sync_info = mybir.SyncInfo(
                        on_wait=[ow[-1]], on_update=list(si.on_update or [])
                    )
                new_insts.append(inst)
            b.instructions[:] = new_insts


def _build(loop_iters=None):
    f32 = mybir.dt.float32
    f32r = mybir.dt.float32r

    nc = bass.Bass("TRN2", target_bir_lowering=False, debug=False,
                   num_devices=NCORES)
    x = nc.dram_tensor("x", [N, E], f32r, kind="ExternalInput").ap()
    xk = nc.dram_tensor("xk", [JB * B, E], f32r, kind="ExternalInput").ap()
    wqt = nc.dram_tensor("wqt", [E, D], f32r, kind="ExternalInput").ap()
    wkt = nc.dram_tensor("wkt", [E, D], f32r, kind="ExternalInput").ap()
    bq = nc.dram_tensor("bq", [D], f32, kind="ExternalInput").ap()
    bk = nc.dram_tensor("bk", [D], f32, kind="ExternalInput").ap()
    out = nc.dram_tensor("out", [N, JB, 3], f32, kind="ExternalOutput").ap()

    with tile.TileContext(nc) as tc:
        with (
            tc.tile_pool(name="wpool", bufs=1) as wpool,
            tc.tile_pool(name="kpool", bufs=1) as kpool,
            tc.tile_pool(name="xin", bufs=2) as xin,
            tc.tile_pool(name="xtst", bufs=2) as xtst,
            tc.tile_pool(name="qpool", bufs=2) as qpool,
            tc.tile_pool(name="spool", bufs=2) as spool,
            tc.tile_pool(name="misc", bufs=1) as misc,
            tc.tile_pool(name="ps_t", bufs=1, space="PSUM") as ps_t,
            tc.tile_pool(name="ps_p", bufs=1, space="PSUM") as ps_p,
            tc.tile_pool(name="ps_s", bufs=2, space="PSUM") as ps_s,
            tc.tile_pool(name="ps_m", bufs=1, space="PSUM") as ps_m,
        ):
            import contextlib
            loop_cm = (tc.For_i(0, loop_iters, 1) if loop_iters
                       else contextlib.nullcontext())

            # ---- weights / constants (outside the timing loop) -----------
            wq_sb = wpool.tile([128, 8, D], f32r, tag="wq")
            wk_sb = wpool.tile([128, 8, D], f32r, tag="wk")
            nc.sync.dma_start(out=wq_sb, in_=wqt.rearrange("(c p) d -> p c d", p=128))
            nc.sync.dma_start(out=wk_sb, in_=wkt.rearrange("(c p) d -> p c d", p=128))
            bq_sb = wpool.tile([128, 4], f32, tag="bq")
            bk_sb = wpool.tile([128, 4], f32, tag="bk")
            nc.sync.dma_start(out=bq_sb, in_=bq.rearrange("(c p) -> p c", p=128))
            nc.sync.dma_start(out=bk_sb, in_=bk.rearrange("(c p) -> p c", p=128))
            ident_f = misc.tile([128, 128], f32, tag="ident_f")
            make_identity(nc, ident_f)
            ident = misc.tile([128, 128], f32r, tag="ident")
            nc.vector.tensor_copy(ident, ident_f)

            kT = kpool.tile([128, 4, JB * B], f32r, tag="kT")   # [dpart, dchunk, n]
            ksum_f = misc.tile([128, 4, JB], f32, tag="ksum_f")
            ksum = misc.tile([128, 4, JB], f32r, tag="ksum")
            scratch = misc.tile([128, B], f32, tag="scratch")

            loop_cm.__enter__()

            def load_span(src_rows):
                """DMA 512 input rows -> [128, 4(tt), 1024(e)] f32r tile."""
                t = xin.tile([128, 4, E], f32r, tag="xspan")
                nc.sync.dma_start(
                    out=t, in_=src_rows.rearrange("(tt p) e -> p tt e", p=128)
                )
                return t

            def transpose_span(xt):
                """[128, 4, 1024] span -> xT staging [128, 8(echunk), 512(n)]."""
                st = xtst.tile([128, 8, B], f32r, tag="xT")
                for half in range(4):
                    tp = ps_t.tile([128, 2, B], f32r, tag="tp")
                    for tt in range(4):
                        for mi in range(2):
                            m = half * 2 + mi
                            nc.tensor.transpose(
                                tp[:, mi, tt * 128:(tt + 1) * 128],
                                xt[:, tt, m * 128:(m + 1) * 128],
                                ident,
                            )
                    nc.scalar.activation(
                        st[:, half * 2:(half + 1) * 2, :], tp, AF.Identity
                    )
                return st

            def project(st, w_sb, b_sb, dst3, n0):
                """xT staging + weights -> dst3[:, dm, n0:n0+512] (+bias)."""
                for dm in range(4):
                    pp = ps_p.tile([128, B], f32, tag="pp")
                    for e in range(8):
                        nc.tensor.matmul(
                            pp,
                            w_sb[:, e, dm * 128:(dm + 1) * 128],
                            st[:, e, :],
                            start=(e == 0),
                            stop=(e == 7),
                        )
                    nc.scalar.activation(
                        dst3[:, dm, n0:n0 + B], pp, AF.Identity,
                        bias=b_sb[:, dm:dm + 1],
                    )

            # ---- phase K: kT for the core's 4 partner groups -------------
            for js in range(JB):
                st = transpose_span(load_span(xk[js * B:(js + 1) * B, :]))
                project(st, wk_sb, bk_sb, kT, js * B)

            # per-group k sums (exact inner mean via linearity); the ACT
            # copy's accum_out gives the per-partition sum for free
            for dm in range(4):
                for j in range(JB):
                    nc.scalar.activation(
                        scratch, kT[:, dm, j * B:(j + 1) * B], AF.Identity,
                        accum_out=ksum_f[:, dm, j:j + 1],
                    )
            nc.vector.tensor_copy(ksum, ksum_f)

            # ---- phase G: per query group, project q and emit stats ------
            for g in range(G):
                st = transpose_span(load_span(x[g * B:(g + 1) * B, :]))
                qg = qpool.tile([128, 4, B], f32r, tag="qg")
                project(st, wq_sb, bq_sb, qg, 0)

                stats = spool.tile([128, 4, JB, 3], f32, tag="stats")
                for qt in range(4):
                    lhs = [qg[:, dc, qt * 128:(qt + 1) * 128] for dc in range(4)]
                    # inner mean: (q . ksum_j) / B
                    mp = ps_m.tile([128, JB], f32, tag="mp")
                    for dc in range(4):
                        nc.tensor.matmul(mp, lhs[dc], ksum[:, dc, :],
                                         start=(dc == 0), stop=(dc == 3))
                    nc.scalar.activation(
                        stats[:, qt, :, 0], mp, AF.Identity, scale=1.0 / B
                    )
                    # inner max / min over each partner group's 512 boxes
                    for jc in range(2):
                        sp = ps_s.tile([128, 2, B], f32, tag="sp")
                        for jj in range(2):
                            j = jc * 2 + jj
                            for dc in range(4):
                                nc.tensor.matmul(
                                    sp[:, jj, :], lhs[dc],
                                    kT[:, dc, j * B:(j + 1) * B],
                                    start=(dc == 0), stop=(dc == 3),
                                )
                        nc.vector.tensor_reduce(
                            stats[:, qt, jc * 2:(jc + 1) * 2, 1], sp,
                            axis=X_AXIS, op=mybir.AluOpType.max,
                        )
                        nc.vector.tensor_reduce(
                            stats[:, qt, jc * 2:(jc + 1) * 2, 2], sp,
                            axis=X_AXIS, op=mybir.AluOpType.min,
                        )
                nc.sync.dma_start(
                    out=out[g * B:(g + 1) * B]
                    .rearrange("(qt p) j m -> p qt j m", p=128),
                    in_=stats,
                )

            loop_cm.__exit__(None, None, None)

    _split_multi_waits(nc)
    return nc


_NC_CACHE = None


def _get_nc():
    global _NC_CACHE
    if _NC_CACHE is None:
        _NC_CACHE = _build()
    return _NC_CACHE


def make_in_maps(x, Wq, bq, Wk, bk):
    x = np.ascontiguousarray(np.asarray(x, dtype=np.float32).reshape(N, E))
    in_maps = []
    for c in range(NCORES):
        h, jb = divmod(c, JB)
        sl = slice(h * D, (h + 1) * D)
        in_maps.append({
            "x": x,
            "xk": np.ascontiguousarray(x[jb * JB * B:(jb + 1) * JB * B, :]),
            "wqt": np.ascontiguousarray(np.asarray(Wq, np.float32)[sl, :].T),
            "wkt": np.ascontiguousarray(np.asarray(Wk, np.float32)[sl, :].T),
            "bq": np.ascontiguousarray(np.asarray(bq, np.float32)[sl]),
            "bk": np.ascontiguousarray(np.asarray(bk, np.float32)[sl]),
        })
    return in_maps


def combine_outputs(core_outs):
    """core_outs: 8 arrays [N, JB, 3] -> final [N, 18].

    Outer reductions over partner groups j != g plus the reference's
    'n m o h -> o (n m h)' output interleave."""
    final = np.empty((N, 18), dtype=np.float32)
    for h in range(2):
        stats = np.concatenate(
            [core_outs[h * JB + jb] for jb in range(JB)], axis=1
        )  # [N, 16(j), 3(m: mean/max/min)]
        for g in range(G):
            rows = slice(g * B, (g + 1) * B)
            sub = np.delete(stats[rows], g, axis=1)  # [B, 15, 3]
            outer = np.stack(
                [sub.mean(1), sub.max(1), sub.min(1)], axis=1
            )  # [B, 3(n: avg/max/min), 3(m)]
            final[rows, h::2] = outer.reshape(B, 9)
    return final


def kernel(x, Wq, bq, Wk, bk, eachimg_selected_box_nums):
    # eachimg_selected_box_nums is uniform (shape [16], all 512) and, like
    # the reference, only its shape matters — the kernel hardcodes G/B.
    nc = _get_nc()
    in_maps = make_in_maps(x, Wq, bq, Wk, bk)
    res = run_bass_kernel_spmd(nc, in_maps, core_ids=list(range(NCORES)))
    return combine_outputs([res.results[c]["out"] for c in range(NCORES)])
